# revision 4
# baseline (speedup 1.0000x reference)
"""RNN-T joint network kernel for 8 Trainium2 NeuronCores — fp8 DoubleRow v2.

Reference computation:
    e = enc_out @ W_enc.T + b_enc                 # [B,T,J]
    d = pred_out @ W_dec.T + b_dec                # [B,U,J]
    joint = tanh(e[:,:,None,:] + d[:,None,:,:])
    out   = joint @ W_out.T + b_out               # [B,T,U,V]

Shapes (hardcoded): B=4, T=256, U=128, D=512, J=640, V=1024.
Sharding: data-parallel over B*T rows; core k handles batch k//2, t-range
(k%2)*128..+128; each core emits its [128,128,1024] f32 output slab.

Algorithm (per core) — linear-residual decomposition so the dominant
[T*U, J] @ [J, V] GEMM runs in fp8-e4m3 DoubleRow mode (2 contraction rows
per PE cell per cycle) while meeting the accuracy budget:

    tanh(x) = ALPHA*x - r(x),    r(x) = ALPHA*x - tanh(x), rms(r) ~ 0.13
    out[t,u,v] = -sum_j r*W[v,j]   (fp8 DR GEMM)
               + A*EW[t,v] + A*DW[u,v] + b_out[v]   (exact compensation)

  - ALPHA is folded into W_enc/b_enc/W_dec/b_dec on the host, so the
    projections produce x' = ALPHA*e, ALPHA*d directly (fp16).
  - GPSIMD builds the lattice x'[j,u] = dP[j,u] + eP[j,t] (5 scalar-adds/t).
  - ACT computes jt = tanh(x'/ALPHA) in ONE full-width op per t (no bias).
  - DVE computes the GEMM operand fp8(x' - jt) in ONE full-width op per t.
  - A*DW + b_out is added on the 2-row-batched PSUM->SBUF drain (DVE,
    tensor_sub with the sign flip: out = GWb - psum).
  - -A*EW[t,:] rides the third DR matmul's spare pair slot: stationary
    slot1 holds (1.0 @p0, 0.0625 @p1, zeros), moving slot1 holds fp8 hi/lo
    rows of -A*EW (lo pre-scaled by 16), accurate to ~2^-8 for free.
"""

import os
import numpy as np

B, T, U, D, J, V = 4, 256, 128, 512, 640, 1024
NCORES = 8
TC = (B * T) // NCORES          # 128 t-rows per core
JC = J // 128                   # 5 j-chunks
DC = D // 128                   # 4 d-chunks
G = 8                           # t-rows per FW-row group
NG = TC // G                    # 16 groups
NB_LAT = 6
NB_JT = 6

ALPHA = 0.62

MAIN_DT_NAME = "fp8dr_v2"

_CACHE = {}


def _build_bass():
    import concourse.mybir as mybir
    import concourse.tile as tile
    import concourse.bacc as bacc

    f32 = mybir.dt.float32
    f16 = mybir.dt.float16
    fp8 = mybir.dt.float8e4
    DR = mybir.MatmulPerfMode.DoubleRow
    Tanh = mybir.ActivationFunctionType.Tanh

    nc = bacc.Bacc("TRN2", debug=False)

    enc_d = nc.dram_tensor("enct", [D, TC], f16, kind="ExternalInput")
    pred_d = nc.dram_tensor("predt", [D, U], f16, kind="ExternalInput")
    wenc_d = nc.dram_tensor("wenct", [D, J], f16, kind="ExternalInput")
    wdec_d = nc.dram_tensor("wdect", [D, J], f16, kind="ExternalInput")
    wo8a_d = nc.dram_tensor("wo8a", [128, 2, V], fp8, kind="ExternalInput")
    wo8b_d = nc.dram_tensor("wo8b", [128, 2, V], fp8, kind="ExternalInput")
    mstat_d = nc.dram_tensor("mstat", [128, 2, G, V], fp8, kind="ExternalInput")
    latc_d = nc.dram_tensor("latc", [128, U], fp8, kind="ExternalInput")
    wo16_d = nc.dram_tensor("wo16", [128, JC, V], f16, kind="ExternalInput")
    benc_d = nc.dram_tensor("bencr", [128, JC], f32, kind="ExternalInput")
    bdec_d = nc.dram_tensor("bdecr", [128, JC], f32, kind="ExternalInput")
    bout_d = nc.dram_tensor("boutr", [128, V], f32, kind="ExternalInput")
    out_d = nc.dram_tensor("out", [TC, U, V], f32, kind="ExternalOutput")

    enc_ap, pred_ap = enc_d.ap(), pred_d.ap()
    wenc_ap, wdec_ap = wenc_d.ap(), wdec_d.ap()
    out_ap = out_d.ap()

    with tile.TileContext(nc) as tc:
        with (
            tc.tile_pool(name="consts", bufs=1) as consts,
            tc.tile_pool(name="proj", bufs=1) as proj,
            tc.tile_pool(name="xpp", bufs=NB_JT) as xpp,
            tc.tile_pool(name="jtp", bufs=NB_JT) as jtp,
            tc.tile_pool(name="latp", bufs=NB_LAT) as latp,
            tc.tile_pool(name="osb", bufs=4) as osbp,
            tc.tile_pool(name="psB", bufs=2, space="PSUM") as psB,
        ):
            # ---- load inputs; projection operands first so PE can start ----
            enc_t, pred_t, wenc_t, wdec_t = [], [], [], []
            for dc in range(DC):
                sl = slice(dc * 128, (dc + 1) * 128)
                a = consts.tile([128, TC], f16, tag=f"enc{dc}")
                nc.sync.dma_start(a[:], enc_ap[sl, :])
                enc_t.append(a)
                p = consts.tile([128, U], f16, tag=f"pred{dc}")
                nc.sync.dma_start(p[:], pred_ap[sl, :])
                pred_t.append(p)
                we = consts.tile([128, J], f16, tag=f"wenc{dc}")
                nc.sync.dma_start(we[:], wenc_ap[sl, :])
                wenc_t.append(we)
                wd = consts.tile([128, J], f16, tag=f"wdec{dc}")
                nc.sync.dma_start(wd[:], wdec_ap[sl, :])
                wdec_t.append(wd)

            benc_t = consts.tile([128, JC], f32, tag="benc")
            nc.sync.dma_start(benc_t[:], benc_d.ap()[:])
            bdec_t = consts.tile([128, JC], f32, tag="bdec")
            nc.sync.dma_start(bdec_t[:], bdec_d.ap()[:])
            wo16_t = consts.tile([128, JC, V], f16, tag="wo16")
            nc.sync.dma_start(wo16_t[:], wo16_d.ap()[:])
            wo8a_t = consts.tile([128, 2, V], fp8, tag="wo8a")
            nc.sync.dma_start(wo8a_t[:], wo8a_d.ap()[:])
            wo8b_t = consts.tile([128, 2, V], fp8, tag="wo8b")
            nc.sync.dma_start(wo8b_t[:], wo8b_d.ap()[:])
            bout_t = consts.tile([128, V], f32, tag="bout")
            nc.sync.dma_start(bout_t[:], bout_d.ap()[:])
            m_t = []
            for mb in range(2):
                m = consts.tile([128, 2, G, V], fp8, tag=f"m{mb}")
                nc.sync.dma_start(m[:], mstat_d.ap()[:])
                m_t.append(m)

            # preload the tanh table set so the main loop doesn't stall
            warm = proj.tile([128, 1], f32, tag="warm")
            nc.scalar.activation(warm[:], benc_t[:, 0:1], Tanh)

            # ---- projections (alpha pre-folded): encP[c][j,t], decP[c][j,u]
            encP, decP = [], []
            for c in range(JC):
                jsl = slice(c * 128, (c + 1) * 128)
                pse = psB.tile([128, TC], f32, tag="ps")
                for dc in range(DC):
                    nc.tensor.matmul(pse[:], wenc_t[dc][:, jsl], enc_t[dc][:],
                                     start=(dc == 0), stop=(dc == DC - 1))
                e = proj.tile([128, TC], f32, tag=f"encP{c}")
                nc.vector.tensor_scalar_add(e[:], pse[:], benc_t[:, c:c + 1])
                encP.append(e)

                psd = psB.tile([128, U], f32, tag="ps")
                for dc in range(DC):
                    nc.tensor.matmul(psd[:], wdec_t[dc][:, jsl], pred_t[dc][:],
                                     start=(dc == 0), stop=(dc == DC - 1))
                d = proj.tile([128, U], f16, tag=f"decP{c}")
                nc.vector.tensor_scalar_add(d[:], psd[:], bdec_t[:, c:c + 1])
                decP.append(d)

            # fp16 copy of encP for the EW GEMM (negligible vs f32 scalar)
            eP16 = proj.tile([128, JC, TC], f16, tag="eP16")
            for c in range(JC):
                nc.vector.tensor_copy(eP16[:, c, :], encP[c][:])

            # ---- FW rows: -A*EW = -(encP' @ WoT), then fp8 hi/lo ----
            psf = psB.tile([128, V], f32, tag="ps")
            for vh in range(2):
                vsl = slice(vh * 512, (vh + 1) * 512)
                for c in range(JC):
                    nc.tensor.matmul(psf[:, vsl], eP16[:, c, :], wo16_t[:, c, vsl],
                                     start=(c == 0), stop=(c == JC - 1))
            fw32 = proj.tile([128, V], f32, tag="fw32")
            nc.vector.tensor_scalar_mul(fw32[:], psf[:], -1.0)
            fwhi = proj.tile([128, V], fp8, tag="fwhi")
            nc.vector.tensor_copy(fwhi[:], fw32[:])
            fwrem = proj.tile([128, V], f32, tag="fwrem")
            nc.vector.tensor_sub(fwrem[:], fw32[:], fwhi[:])
            fwlo = proj.tile([128, V], fp8, tag="fwlo")
            nc.vector.tensor_scalar_mul(fwlo[:], fwrem[:], 16.0)

            # ---- GWb2 = [A*DW + b_out] twice (for 2-row drains) ----
            psg = psB.tile([128, V], f32, tag="ps")
            for vh in range(2):
                vsl = slice(vh * 512, (vh + 1) * 512)
                for c in range(JC):
                    nc.tensor.matmul(psg[:, vsl], decP[c][:], wo16_t[:, c, vsl],
                                     start=(c == 0), stop=(c == JC - 1))
            gwb2 = proj.tile([128, 2, V], f32, tag="gwb2")
            nc.vector.tensor_add(gwb2[:, 0, :], psg[:], bout_t[:])
            nc.vector.tensor_copy(gwb2[:, 1, :], gwb2[:, 0, :])

            # ---- pre-init lattice pool: slot 5 = FW stationary consts ----
            for _ in range(NB_LAT):
                lt = latp.tile([128, 6, U], fp8, tag="lat")
                nc.sync.dma_start(lt[:, 5, :], latc_d.ap()[:])

            # FW hi/lo rows for group 0 into m buffer 0
            nc.sync.dma_start(m_t[0][0:1, 1, :, :], fwhi[0:G, :])
            nc.sync.dma_start(m_t[0][1:2, 1, :, :], fwlo[0:G, :])

            # ---- main loop ----
            for g in range(NG):
                if g + 1 < NG:
                    mnext = m_t[(g + 1) % 2]
                    rs = slice((g + 1) * G, (g + 2) * G)
                    nc.sync.dma_start(mnext[0:1, 1, :, :], fwhi[rs, :])
                    nc.sync.dma_start(mnext[1:2, 1, :, :], fwlo[rs, :])
                mcur = m_t[g % 2]
                for p in range(G // 2):
                    ps2 = psB.tile([128, 2 * V], f32, tag="ps")
                    lats = []
                    for i2 in range(2):
                        t = g * G + 2 * p + i2
                        xp = xpp.tile([128, JC * U], f16, tag="xp")
                        for c in range(JC):
                            nc.gpsimd.tensor_scalar_add(
                                xp[:, c * U:(c + 1) * U], decP[c][:],
                                encP[c][:, t:t + 1])
                        jt = jtp.tile([128, JC * U], f16, tag="jt")
                        nc.scalar.activation(jt[:], xp[:], Tanh,
                                             scale=1.0 / ALPHA)
                        lat = latp.tile([128, 6, U], fp8, tag="lat")
                        nc.vector.tensor_sub(lat[:, 0:5, :], xp[:], jt[:])
                        lats.append(lat)

                    for i2 in range(2):
                        i = 2 * p + i2
                        lat = lats[i2]
                        for c3 in range(3):
                            lhs = lat[:, 2 * c3:2 * c3 + 2, :]
                            for vh in range(2):
                                sl = slice(i2 * V + vh * 512,
                                           i2 * V + (vh + 1) * 512)
                                vsl = slice(vh * 512, (vh + 1) * 512)
                                rhs = (wo8a_t[:, :, vsl] if c3 == 0 else
                                       wo8b_t[:, :, vsl] if c3 == 1 else
                                       mcur[:, :, i, vsl])
                                nc.tensor.matmul(ps2[:, sl], lhs, rhs,
                                                 start=(c3 == 0),
                                                 stop=(c3 == 2),
                                                 perf_mode=DR)

                    t0 = g * G + 2 * p
                    osb2 = osbp.tile([128, 2 * V], f32, tag="osb")
                    if g == NG - 1 and p == G // 2 - 1:
                        for q in range(4):
                            qsl = slice(q * 512, (q + 1) * 512)
                            hsl = slice((q % 2) * 512, (q % 2) * 512 + 512)
                            nc.vector.tensor_sub(osb2[:, qsl],
                                                 gwb2[:, q // 2, hsl],
                                                 ps2[:, qsl])
                            nc.sync.dma_start(out_ap[t0 + q // 2][:, hsl],
                                              osb2[:, qsl])
                    else:
                        nc.vector.tensor_sub(osb2[:], gwb2[:], ps2[:])
                        nc.sync.dma_start(out_ap[t0], osb2[:, 0:V])
                        nc.sync.dma_start(out_ap[t0 + 1], osb2[:, V:2 * V])

    nc.compile()
    return nc


def _host_prep(enc_out, pred_out, W_enc, b_enc, W_dec, b_dec, W_out, b_out):
    import concourse.mybir as mybir
    np_f16 = np.dtype(np.float16)
    np_fp8 = np.dtype(mybir.dt.np(mybir.dt.float8e4))

    wencT = np.ascontiguousarray(
        (np.asarray(W_enc, np.float32) * ALPHA).T).astype(np_f16)
    wdecT = np.ascontiguousarray(
        (np.asarray(W_dec, np.float32) * ALPHA).T).astype(np_f16)
    woT = np.ascontiguousarray(np.asarray(W_out, np.float32).T)      # [J, V]

    wo16 = np.ascontiguousarray(
        woT.reshape(JC, 128, V).transpose(1, 0, 2)).astype(np_f16)   # [128,JC,V]
    wo8 = woT.astype(np_fp8)                                         # [J, V]
    wo8a = np.ascontiguousarray(
        wo8[0:256].reshape(2, 128, V).transpose(1, 0, 2))            # [128,2,V]
    wo8b = np.ascontiguousarray(
        wo8[256:512].reshape(2, 128, V).transpose(1, 0, 2))
    mstat = np.zeros((128, 2, G, V), np_fp8)
    mstat[:, 0, :, :] = wo8[512:640][:, None, :]                     # W c4 x G
    latc = np.zeros((128, U), np_fp8)
    latc[0, :] = 1.0
    latc[1, :] = 0.0625

    bencr = np.ascontiguousarray(
        (np.asarray(b_enc, np.float32) * ALPHA).reshape(JC, 128).T)
    bdecr = np.ascontiguousarray(
        (np.asarray(b_dec, np.float32) * ALPHA).reshape(JC, 128).T)
    boutr = np.ascontiguousarray(
        np.broadcast_to(np.asarray(b_out, np.float32), (128, V)))

    in_maps = []
    for k in range(NCORES):
        b, th = k // 2, (k % 2) * TC
        encT = np.ascontiguousarray(
            np.asarray(enc_out[b, th:th + TC], np.float32).T).astype(np_f16)
        predT = np.ascontiguousarray(
            np.asarray(pred_out[b], np.float32).T).astype(np_f16)
        in_maps.append({
            "enct": encT, "predt": predT, "wenct": wencT, "wdect": wdecT,
            "wo8a": wo8a, "wo8b": wo8b, "mstat": mstat, "latc": latc,
            "wo16": wo16, "bencr": bencr, "bdecr": bdecr, "boutr": boutr,
        })
    return in_maps


def kernel(enc_out, pred_out, W_enc, b_enc, W_dec, b_dec, W_out, b_out):
    from concourse import bass_utils

    if "nc" not in _CACHE:
        _CACHE["nc"] = _build_bass()
    nc = _CACHE["nc"]

    in_maps = _host_prep(enc_out, pred_out, W_enc, b_enc, W_dec, b_dec,
                         W_out, b_out)

    trace = bool(int(os.environ.get("TRNK_PROFILE", "0")))
    res = bass_utils.run_bass_kernel_spmd(
        nc, in_maps, core_ids=list(range(NCORES)), trace=trace)
    kernel.last_exec_ns = res.exec_time_ns

    full = np.empty((B, T, U, V), np.float32)
    for k in range(NCORES):
        b, th = k // 2, (k % 2) * TC
        full[b, th:th + TC] = res.results[k]["out"]
    return full


kernel.last_exec_ns = None


# revision 11
# speedup vs baseline: 3.8003x; 3.8003x over previous
"""RNN-T joint network kernel for 8 Trainium2 NeuronCores — hybrid fp8/fp16 v3.

Reference computation:
    e = enc_out @ W_enc.T + b_enc                 # [B,T,J]
    d = pred_out @ W_dec.T + b_dec                # [B,U,J]
    joint = tanh(e[:,:,None,:] + d[:,None,:,:])
    out   = joint @ W_out.T + b_out               # [B,T,U,V]

Shapes (hardcoded): B=4, T=256, U=128, D=512, J=640, V=1024.
Sharding: data-parallel over B*T rows; core k handles batch k//2, t-range
(k%2)*128..+128; each core emits its [128,128,1024] f32 output slab.

The dominant [T*U,J]@[J,V] GEMM is PE-bound in fp16 (~276us/core) while the
fp8-DoubleRow formulation (2 contraction rows/cell/cycle) shifts ~40% of the
PE work onto the vector engines.  Neither engine class can absorb the whole
problem, so t-groups are SPLIT: NF_FP8 of 16 groups run the fp8 path, the
rest run the plain fp16 path, balancing PE against DVE/ACT.

fp8 path (linear-residual decomposition, ALPHA folded into host weights):
    tanh(x) = A*x - r(x),  r = A*x - tanh(x), rms(r)~0.13 -> fp8-safe
    psum    = sum_j fp8(r)*fp8(W) - A*EW[t,:]      (3 DR matmuls)
    out     = (A*DW[u,:] + b_out) - psum           (DVE drain)
  - DVE builds x' = dP+eP[:,t] (5 scalar-adds) and r at fp16 (2x mode),
    ACT does one full-width tanh + one fp16->fp8 cast per t.
  - -A*EW[t,:] rides DR matmul 3's spare pair slot (fp8 hi/lo rows).

fp16 path (per t): ACT tanh with per-partition e-bias (5 ops), 10 fp16
matmuls, drain adds b_out only.
"""

import os
import numpy as np

B, T, U, D, J, V = 4, 256, 128, 512, 640, 1024
NCORES = 8
TC = (B * T) // NCORES          # 128 t-rows per core
JC = J // 128                   # 5 j-chunks
DC = D // 128                   # 4 d-chunks
G = 8                           # t-rows per group
NG = TC // G                    # 16 groups
NF_FP8 = 9                      # first NF_FP8 groups use the fp8 path
NB_LAT = 6
NB_JT = 4

ALPHA = 0.62

MAIN_DT_NAME = "hybrid_v3"

_CACHE = {}


def _build_bass():
    import concourse.mybir as mybir
    import concourse.tile as tile
    import concourse.bacc as bacc

    f32 = mybir.dt.float32
    f16 = mybir.dt.float16
    fp8 = mybir.dt.float8e4
    DR = mybir.MatmulPerfMode.DoubleRow
    Tanh = mybir.ActivationFunctionType.Tanh
    Copy = mybir.ActivationFunctionType.Copy

    nc = bacc.Bacc("TRN2", debug=False)

    enc_d = nc.dram_tensor("enct", [D, TC], f16, kind="ExternalInput")
    pred_d = nc.dram_tensor("predt", [D, U], f16, kind="ExternalInput")
    wenc_d = nc.dram_tensor("wenct", [D, J], f16, kind="ExternalInput")
    wdec_d = nc.dram_tensor("wdect", [D, J], f16, kind="ExternalInput")
    wo8a_d = nc.dram_tensor("wo8a", [128, 2, V], fp8, kind="ExternalInput")
    wo8b_d = nc.dram_tensor("wo8b", [128, 2, V], fp8, kind="ExternalInput")
    mstat_d = nc.dram_tensor("mstat", [128, 2, G, V], fp8, kind="ExternalInput")
    latc_d = nc.dram_tensor("latc", [128, U], fp8, kind="ExternalInput")
    wo16_d = nc.dram_tensor("wo16", [128, JC, V], f16, kind="ExternalInput")
    benc_d = nc.dram_tensor("bencr", [128, JC], f32, kind="ExternalInput")
    bencu_d = nc.dram_tensor("bencu", [128, JC], f32, kind="ExternalInput")
    bdec_d = nc.dram_tensor("bdecr", [128, JC], f32, kind="ExternalInput")
    bout_d = nc.dram_tensor("boutr", [128, 2, V], f32, kind="ExternalInput")
    out_d = nc.dram_tensor("out", [TC, U, V], f32, kind="ExternalOutput")

    enc_ap, pred_ap = enc_d.ap(), pred_d.ap()
    wenc_ap, wdec_ap = wenc_d.ap(), wdec_d.ap()
    out_ap = out_d.ap()

    TF8 = NF_FP8 * G            # t's on the fp8 path

    with tile.TileContext(nc) as tc:
        with (
            tc.tile_pool(name="consts", bufs=1) as consts,
            tc.tile_pool(name="proj", bufs=1) as proj,
            tc.tile_pool(name="xpp", bufs=NB_JT) as xpp,
            tc.tile_pool(name="jtp", bufs=NB_JT) as jtp,
            tc.tile_pool(name="rsp", bufs=NB_JT) as rsp,
            tc.tile_pool(name="latp", bufs=NB_LAT) as latp,
            tc.tile_pool(name="jbp", bufs=NB_JT) as jbp,
            tc.tile_pool(name="osb", bufs=4) as osbp,
            tc.tile_pool(name="psB", bufs=2, space="PSUM") as psB,
        ):
            # ---- load inputs; projection operands first so PE can start ----
            enc_t, pred_t, wenc_t, wdec_t = [], [], [], []
            for dc in range(DC):
                sl = slice(dc * 128, (dc + 1) * 128)
                a = consts.tile([128, TC], f16, tag=f"enc{dc}")
                nc.sync.dma_start(a[:], enc_ap[sl, :])
                enc_t.append(a)
                p = consts.tile([128, U], f16, tag=f"pred{dc}")
                nc.sync.dma_start(p[:], pred_ap[sl, :])
                pred_t.append(p)
                we = consts.tile([128, J], f16, tag=f"wenc{dc}")
                nc.sync.dma_start(we[:], wenc_ap[sl, :])
                wenc_t.append(we)
                wd = consts.tile([128, J], f16, tag=f"wdec{dc}")
                nc.sync.dma_start(wd[:], wdec_ap[sl, :])
                wdec_t.append(wd)

            benc_t = consts.tile([128, JC], f32, tag="benc")
            nc.sync.dma_start(benc_t[:], benc_d.ap()[:])
            bencu_t = consts.tile([128, JC], f32, tag="bencu")
            nc.sync.dma_start(bencu_t[:], bencu_d.ap()[:])
            bdec_t = consts.tile([128, JC], f32, tag="bdec")
            nc.sync.dma_start(bdec_t[:], bdec_d.ap()[:])
            wo16_t = consts.tile([128, JC, V], f16, tag="wo16")
            nc.sync.dma_start(wo16_t[:], wo16_d.ap()[:])
            wo8a_t = consts.tile([128, 2, V], fp8, tag="wo8a")
            nc.sync.dma_start(wo8a_t[:], wo8a_d.ap()[:])
            wo8b_t = consts.tile([128, 2, V], fp8, tag="wo8b")
            nc.sync.dma_start(wo8b_t[:], wo8b_d.ap()[:])
            bout_t = consts.tile([128, 2, V], f32, tag="bout")
            nc.sync.dma_start(bout_t[:], bout_d.ap()[:])
            m_t = []
            for mb in range(2):
                m = consts.tile([128, 2, G, V], fp8, tag=f"m{mb}")
                nc.sync.dma_start(m[:], mstat_d.ap()[:])
                m_t.append(m)

            # preload the tanh table set
            warm = proj.tile([128, 1], f32, tag="warm")
            nc.scalar.activation(warm[:], benc_t[:, 0:1], Tanh)

            # ---- projections. fp8 path needs alpha-scaled eP/dP; fp16 path
            # needs unscaled.  Scale is folded on host into W (alpha) so the
            # psum holds alpha*proj; unscaled decP/encP derived with 1/alpha.
            encP, decP, encPu, decPu = [], [], [], []
            for c in range(JC):
                jsl = slice(c * 128, (c + 1) * 128)
                pse = psB.tile([128, TC], f32, tag="ps")
                for dc in range(DC):
                    nc.tensor.matmul(pse[:], wenc_t[dc][:, jsl], enc_t[dc][:],
                                     start=(dc == 0), stop=(dc == DC - 1))
                e = proj.tile([128, TC], f32, tag=f"encP{c}")
                nc.vector.tensor_scalar_add(e[:], pse[:], benc_t[:, c:c + 1])
                encP.append(e)
                eu = proj.tile([128, TC], f32, tag=f"encPu{c}")
                nc.vector.tensor_scalar(eu[:], pse[:], benc_t[:, c:c + 1],
                                        1.0 / ALPHA,
                                        mybir.AluOpType.add,
                                        mybir.AluOpType.mult)
                encPu.append(eu)

                psd = psB.tile([128, U], f32, tag="ps")
                for dc in range(DC):
                    nc.tensor.matmul(psd[:], wdec_t[dc][:, jsl], pred_t[dc][:],
                                     start=(dc == 0), stop=(dc == DC - 1))
                d = proj.tile([128, U], f16, tag=f"decP{c}")
                nc.vector.tensor_scalar_add(d[:], psd[:], bdec_t[:, c:c + 1])
                decP.append(d)
                du = proj.tile([128, U], f16, tag=f"decPu{c}")
                nc.vector.tensor_scalar(du[:], psd[:], bdec_t[:, c:c + 1],
                                        1.0 / ALPHA,
                                        mybir.AluOpType.add,
                                        mybir.AluOpType.mult)
                decPu.append(du)

            # fp16 copy of scaled encP for the EW GEMM
            eP16 = proj.tile([128, JC, TC], f16, tag="eP16")
            for c in range(JC):
                nc.vector.tensor_copy(eP16[:, c, :], encP[c][:])

            # ---- FW rows: -A*EW over fp8-path groups, fp8 hi/lo ----
            psf = psB.tile([128, V], f32, tag="ps")
            for vh in range(2):
                vsl = slice(vh * 512, (vh + 1) * 512)
                for c in range(JC):
                    nc.tensor.matmul(psf[:, vsl], eP16[:, c, :], wo16_t[:, c, vsl],
                                     start=(c == 0), stop=(c == JC - 1))
            fw32 = proj.tile([128, V], f32, tag="fw32")
            nc.vector.tensor_scalar_mul(fw32[:], psf[:], -1.0)
            fwhi = proj.tile([128, V], fp8, tag="fwhi")
            nc.vector.tensor_copy(fwhi[:], fw32[:])
            fwrem = proj.tile([128, V], f32, tag="fwrem")
            nc.vector.tensor_sub(fwrem[:], fw32[:], fwhi[:])
            fwlo = proj.tile([128, V], fp8, tag="fwlo")
            nc.vector.tensor_scalar_mul(fwlo[:], fwrem[:], 16.0)

            # ---- GWb2 = [A*DW + b_out] twice (drain operand, fp8 path) ----
            psg = psB.tile([128, V], f32, tag="ps")
            for vh in range(2):
                vsl = slice(vh * 512, (vh + 1) * 512)
                for c in range(JC):
                    nc.tensor.matmul(psg[:, vsl], decP[c][:], wo16_t[:, c, vsl],
                                     start=(c == 0), stop=(c == JC - 1))
            gwb2 = proj.tile([128, 2, V], f32, tag="gwb2")
            nc.vector.tensor_add(gwb2[:, 0, :], psg[:], bout_t[:, 0, :])
            nc.vector.tensor_copy(gwb2[:, 1, :], gwb2[:, 0, :])

            # ---- pre-init lattice pool: slot 5 of each half holds the FW
            # stationary consts (1.0@p0, 1/16@p1, zeros) ----
            for _ in range(NB_LAT):
                lt = latp.tile([128, 2, 6, U], fp8, tag="lat")
                nc.sync.dma_start(lt[:, 0, 5, :], latc_d.ap()[:])
                nc.sync.dma_start(lt[:, 1, 5, :], latc_d.ap()[:])

            nc.sync.dma_start(m_t[0][0:1, 1, :, :], fwhi[0:G, :])
            nc.sync.dma_start(m_t[0][1:2, 1, :, :], fwlo[0:G, :])

            # ---- main loop ----
            for g in range(NG):
                fp8_path = g < NF_FP8
                if g + 1 < NF_FP8:
                    mnext = m_t[(g + 1) % 2]
                    rs = slice((g + 1) * G, (g + 2) * G)
                    nc.sync.dma_start(mnext[0:1, 1, :, :], fwhi[rs, :])
                    nc.sync.dma_start(mnext[1:2, 1, :, :], fwlo[rs, :])
                mcur = m_t[g % 2]
                for p in range(G // 2):
                    ps2 = psB.tile([128, 2 * V], f32, tag="ps")
                    if fp8_path:
                        xp = xpp.tile([128, 2, JC * U], f16, tag="xp")
                        for i2 in range(2):
                            t = g * G + 2 * p + i2
                            for c in range(JC):
                                nc.vector.tensor_scalar_add(
                                    xp[:, i2, c * U:(c + 1) * U], decP[c][:],
                                    encP[c][:, t:t + 1])
                        jt = jtp.tile([128, 2 * JC * U], f16, tag="jt")
                        nc.scalar.activation(jt[:], xp[:], Tanh,
                                             scale=1.0 / ALPHA)
                        rs16 = rsp.tile([128, 2 * JC * U], f16, tag="rs")
                        nc.vector.tensor_sub(rs16[:], xp[:], jt[:])
                        lat = latp.tile([128, 2, 6, U], fp8, tag="lat")
                        nc.scalar.activation(lat[:, :, 0:5, :], rs16[:], Copy)
                        for i2 in range(2):
                            i = 2 * p + i2
                            for c3 in range(3):
                                lhs = lat[:, i2, 2 * c3:2 * c3 + 2, :]
                                for vh in range(2):
                                    sl = slice(i2 * V + vh * 512,
                                               i2 * V + (vh + 1) * 512)
                                    vsl = slice(vh * 512, (vh + 1) * 512)
                                    rhs = (wo8a_t[:, :, vsl] if c3 == 0 else
                                           wo8b_t[:, :, vsl] if c3 == 1 else
                                           mcur[:, :, i, vsl])
                                    nc.tensor.matmul(ps2[:, sl], lhs, rhs,
                                                     start=(c3 == 0),
                                                     stop=(c3 == 2),
                                                     perf_mode=DR)
                    else:
                        jbs = []
                        for i2 in range(2):
                            t = g * G + 2 * p + i2
                            jb = jbp.tile([128, JC * U], f16, tag="jb")
                            for c in range(JC):
                                nc.scalar.activation(
                                    jb[:, c * U:(c + 1) * U], decPu[c][:],
                                    Tanh, bias=encPu[c][:, t:t + 1])
                            jbs.append(jb)
                        for i2 in range(2):
                            jb = jbs[i2]
                            for c in range(JC):
                                for vh in range(2):
                                    sl = slice(i2 * V + vh * 512,
                                               i2 * V + (vh + 1) * 512)
                                    vsl = slice(vh * 512, (vh + 1) * 512)
                                    nc.tensor.matmul(
                                        ps2[:, sl], jb[:, c * U:(c + 1) * U],
                                        wo16_t[:, c, vsl],
                                        start=(c == 0), stop=(c == JC - 1))

                    t0 = g * G + 2 * p
                    osb2 = osbp.tile([128, 2 * V], f32, tag="osb")
                    last = (g == NG - 1 and p == G // 2 - 1)
                    if fp8_path:
                        nc.vector.tensor_sub(osb2[:], gwb2[:], ps2[:])
                    else:
                        nc.vector.tensor_add(osb2[:], ps2[:], bout_t[:])
                    if last:
                        for q in range(4):
                            qsl = slice(q * 512, (q + 1) * 512)
                            hsl = slice((q % 2) * 512, (q % 2) * 512 + 512)
                            nc.sync.dma_start(out_ap[t0 + q // 2][:, hsl],
                                              osb2[:, qsl])
                    else:
                        nc.sync.dma_start(out_ap[t0], osb2[:, 0:V])
                        nc.sync.dma_start(out_ap[t0 + 1], osb2[:, V:2 * V])

    nc.compile()
    return nc


def _host_prep(enc_out, pred_out, W_enc, b_enc, W_dec, b_dec, W_out, b_out):
    import concourse.mybir as mybir
    np_f16 = np.dtype(np.float16)
    np_fp8 = np.dtype(mybir.dt.np(mybir.dt.float8e4))

    wencT = np.ascontiguousarray(
        (np.asarray(W_enc, np.float32) * ALPHA).T).astype(np_f16)
    wdecT = np.ascontiguousarray(
        (np.asarray(W_dec, np.float32) * ALPHA).T).astype(np_f16)
    woT = np.ascontiguousarray(np.asarray(W_out, np.float32).T)      # [J, V]

    wo16 = np.ascontiguousarray(
        woT.reshape(JC, 128, V).transpose(1, 0, 2)).astype(np_f16)   # [128,JC,V]
    wo8 = woT.astype(np_fp8)                                         # [J, V]
    wo8a = np.ascontiguousarray(
        wo8[0:256].reshape(2, 128, V).transpose(1, 0, 2))            # [128,2,V]
    wo8b = np.ascontiguousarray(
        wo8[256:512].reshape(2, 128, V).transpose(1, 0, 2))
    mstat = np.zeros((128, 2, G, V), np_fp8)
    mstat[:, 0, :, :] = wo8[512:640][:, None, :]                     # W c4 x G
    latc = np.zeros((128, U), np_fp8)
    latc[0, :] = 1.0
    latc[1, :] = 0.0625

    bencr = np.ascontiguousarray(
        (np.asarray(b_enc, np.float32) * ALPHA).reshape(JC, 128).T)
    bencu = np.ascontiguousarray(
        np.asarray(b_enc, np.float32).reshape(JC, 128).T)
    bdecr = np.ascontiguousarray(
        (np.asarray(b_dec, np.float32) * ALPHA).reshape(JC, 128).T)
    boutr = np.ascontiguousarray(np.broadcast_to(
        np.asarray(b_out, np.float32), (128, 2, V)))

    in_maps = []
    for k in range(NCORES):
        b, th = k // 2, (k % 2) * TC
        encT = np.ascontiguousarray(
            np.asarray(enc_out[b, th:th + TC], np.float32).T).astype(np_f16)
        predT = np.ascontiguousarray(
            np.asarray(pred_out[b], np.float32).T).astype(np_f16)
        in_maps.append({
            "enct": encT, "predt": predT, "wenct": wencT, "wdect": wdecT,
            "wo8a": wo8a, "wo8b": wo8b, "mstat": mstat, "latc": latc,
            "wo16": wo16, "bencr": bencr, "bencu": bencu, "bdecr": bdecr,
            "boutr": boutr,
        })
    return in_maps


def kernel(enc_out, pred_out, W_enc, b_enc, W_dec, b_dec, W_out, b_out):
    from concourse import bass_utils

    if "nc" not in _CACHE:
        _CACHE["nc"] = _build_bass()
    nc = _CACHE["nc"]

    in_maps = _host_prep(enc_out, pred_out, W_enc, b_enc, W_dec, b_dec,
                         W_out, b_out)

    trace = bool(int(os.environ.get("TRNK_PROFILE", "0")))
    res = bass_utils.run_bass_kernel_spmd(
        nc, in_maps, core_ids=list(range(NCORES)), trace=trace)
    kernel.last_exec_ns = res.exec_time_ns

    full = np.empty((B, T, U, V), np.float32)
    for k in range(NCORES):
        b, th = k // 2, (k % 2) * TC
        full[b, th:th + TC] = res.results[k]["out"]
    return full


kernel.last_exec_ns = None


# revision 17
# speedup vs baseline: 3.8460x; 1.0120x over previous
"""RNN-T joint network kernel for 8 Trainium2 NeuronCores — hybrid fp8/fp16 v3.

Reference computation:
    e = enc_out @ W_enc.T + b_enc                 # [B,T,J]
    d = pred_out @ W_dec.T + b_dec                # [B,U,J]
    joint = tanh(e[:,:,None,:] + d[:,None,:,:])
    out   = joint @ W_out.T + b_out               # [B,T,U,V]

Shapes (hardcoded): B=4, T=256, U=128, D=512, J=640, V=1024.
Sharding: data-parallel over B*T rows; core k handles batch k//2, t-range
(k%2)*128..+128; each core emits its [128,128,1024] f32 output slab.

The dominant [T*U,J]@[J,V] GEMM is PE-bound in fp16 (~276us/core) while the
fp8-DoubleRow formulation (2 contraction rows/cell/cycle) shifts ~40% of the
PE work onto the vector engines.  Neither engine class can absorb the whole
problem, so t-groups are SPLIT: NF_FP8 of 16 groups run the fp8 path, the
rest run the plain fp16 path, balancing PE against DVE/ACT.

fp8 path (linear-residual decomposition, ALPHA folded into host weights):
    tanh(x) = A*x - r(x),  r = A*x - tanh(x), rms(r)~0.13 -> fp8-safe
    psum    = sum_j fp8(r)*fp8(W) - A*EW[t,:]      (3 DR matmuls)
    out     = (A*DW[u,:] + b_out) - psum           (DVE drain)
  - DVE builds x' = dP+eP[:,t] (5 scalar-adds) and r at fp16 (2x mode),
    ACT does one full-width tanh + one fp16->fp8 cast per t.
  - -A*EW[t,:] rides DR matmul 3's spare pair slot (fp8 hi/lo rows).

fp16 path (per t): ACT tanh with per-partition e-bias (5 ops), 10 fp16
matmuls, drain adds b_out only.
"""

import os
import numpy as np

B, T, U, D, J, V = 4, 256, 128, 512, 640, 1024
NCORES = 8
TC = (B * T) // NCORES          # 128 t-rows per core
JC = J // 128                   # 5 j-chunks
DC = D // 128                   # 4 d-chunks
G = 8                           # t-rows per group
NG = TC // G                    # 16 groups
NF_FP8 = 10                     # first NF_FP8 groups use the fp8 path
NB_LAT = 6
NB_JT = 4

ALPHA = 0.62

MAIN_DT_NAME = "hybrid_v3"

_CACHE = {}


def _build_bass():
    import concourse.mybir as mybir
    import concourse.tile as tile
    import concourse.bacc as bacc

    f32 = mybir.dt.float32
    f16 = mybir.dt.float16
    fp8 = mybir.dt.float8e4
    DR = mybir.MatmulPerfMode.DoubleRow
    Tanh = mybir.ActivationFunctionType.Tanh
    Copy = mybir.ActivationFunctionType.Copy

    nc = bacc.Bacc("TRN2", debug=False)

    enc_d = nc.dram_tensor("enct", [D, TC], f16, kind="ExternalInput")
    pred_d = nc.dram_tensor("predt", [D, U], f16, kind="ExternalInput")
    wenc_d = nc.dram_tensor("wenct", [D, J], f16, kind="ExternalInput")
    wdec_d = nc.dram_tensor("wdect", [D, J], f16, kind="ExternalInput")
    wo8a_d = nc.dram_tensor("wo8a", [128, 2, V], fp8, kind="ExternalInput")
    wo8b_d = nc.dram_tensor("wo8b", [128, 2, V], fp8, kind="ExternalInput")
    mstat_d = nc.dram_tensor("mstat", [128, 2, G, V], fp8, kind="ExternalInput")
    latc_d = nc.dram_tensor("latc", [128, U], fp8, kind="ExternalInput")
    wo16_d = nc.dram_tensor("wo16", [128, JC, V], f16, kind="ExternalInput")
    benc_d = nc.dram_tensor("bencr", [128, JC], f32, kind="ExternalInput")
    bencu_d = nc.dram_tensor("bencu", [128, JC], f32, kind="ExternalInput")
    bdec_d = nc.dram_tensor("bdecr", [128, JC], f32, kind="ExternalInput")
    bout_d = nc.dram_tensor("boutr", [128, 2, V], f32, kind="ExternalInput")
    out_d = nc.dram_tensor("out", [TC, U, V], f32, kind="ExternalOutput")

    enc_ap, pred_ap = enc_d.ap(), pred_d.ap()
    wenc_ap, wdec_ap = wenc_d.ap(), wdec_d.ap()
    out_ap = out_d.ap()

    TF8 = NF_FP8 * G            # t's on the fp8 path

    with tile.TileContext(nc) as tc:
        with (
            tc.tile_pool(name="consts", bufs=1) as consts,
            tc.tile_pool(name="proj", bufs=1) as proj,
            tc.tile_pool(name="xpp", bufs=NB_JT) as xpp,
            tc.tile_pool(name="jtp", bufs=NB_JT) as jtp,
            tc.tile_pool(name="rsp", bufs=NB_JT) as rsp,
            tc.tile_pool(name="latp", bufs=NB_LAT) as latp,
            tc.tile_pool(name="jbp", bufs=NB_JT) as jbp,
            tc.tile_pool(name="osb", bufs=6) as osbp,
            tc.tile_pool(name="psB", bufs=4, space="PSUM") as psB,
        ):
            # ---- load inputs; projection operands first so PE can start ----
            enc_t, pred_t, wenc_t, wdec_t = [], [], [], []
            for dc in range(DC):
                sl = slice(dc * 128, (dc + 1) * 128)
                a = consts.tile([128, TC], f16, tag=f"enc{dc}")
                nc.sync.dma_start(a[:], enc_ap[sl, :])
                enc_t.append(a)
                p = consts.tile([128, U], f16, tag=f"pred{dc}")
                nc.sync.dma_start(p[:], pred_ap[sl, :])
                pred_t.append(p)
                we = consts.tile([128, J], f16, tag=f"wenc{dc}")
                nc.sync.dma_start(we[:], wenc_ap[sl, :])
                wenc_t.append(we)
                wd = consts.tile([128, J], f16, tag=f"wdec{dc}")
                nc.sync.dma_start(wd[:], wdec_ap[sl, :])
                wdec_t.append(wd)

            benc_t = consts.tile([128, JC], f32, tag="benc")
            nc.sync.dma_start(benc_t[:], benc_d.ap()[:])
            bencu_t = consts.tile([128, JC], f32, tag="bencu")
            nc.sync.dma_start(bencu_t[:], bencu_d.ap()[:])
            bdec_t = consts.tile([128, JC], f32, tag="bdec")
            nc.sync.dma_start(bdec_t[:], bdec_d.ap()[:])
            wo16_t = consts.tile([128, JC, V], f16, tag="wo16")
            nc.sync.dma_start(wo16_t[:], wo16_d.ap()[:])
            wo8a_t = consts.tile([128, 2, V], fp8, tag="wo8a")
            nc.sync.dma_start(wo8a_t[:], wo8a_d.ap()[:])
            wo8b_t = consts.tile([128, 2, V], fp8, tag="wo8b")
            nc.sync.dma_start(wo8b_t[:], wo8b_d.ap()[:])
            bout_t = consts.tile([128, 2, V], f32, tag="bout")
            nc.sync.dma_start(bout_t[:], bout_d.ap()[:])
            m_t = []
            for mb in range(2):
                m = consts.tile([128, 2, G, V], fp8, tag=f"m{mb}")
                nc.sync.dma_start(m[:], mstat_d.ap()[:])
                m_t.append(m)

            # preload the tanh table set
            warm = proj.tile([128, 1], f32, tag="warm")
            nc.scalar.activation(warm[:], benc_t[:, 0:1], Tanh)

            # ---- projections. fp8 path needs alpha-scaled eP/dP; fp16 path
            # needs unscaled.  Scale is folded on host into W (alpha) so the
            # psum holds alpha*proj; unscaled decP/encP derived with 1/alpha.
            encP, decP, encPu, decPu = [], [], [], []
            for c in range(JC):
                jsl = slice(c * 128, (c + 1) * 128)
                pse = psB.tile([128, TC], f32, tag="ps")
                for dc in range(DC):
                    nc.tensor.matmul(pse[:], wenc_t[dc][:, jsl], enc_t[dc][:],
                                     start=(dc == 0), stop=(dc == DC - 1))
                e = proj.tile([128, TC], f32, tag=f"encP{c}")
                nc.vector.tensor_scalar_add(e[:], pse[:], benc_t[:, c:c + 1])
                encP.append(e)
                eu = proj.tile([128, TC], f32, tag=f"encPu{c}")
                nc.vector.tensor_scalar(eu[:], pse[:], benc_t[:, c:c + 1],
                                        1.0 / ALPHA,
                                        mybir.AluOpType.add,
                                        mybir.AluOpType.mult)
                encPu.append(eu)

                psd = psB.tile([128, U], f32, tag="ps")
                for dc in range(DC):
                    nc.tensor.matmul(psd[:], wdec_t[dc][:, jsl], pred_t[dc][:],
                                     start=(dc == 0), stop=(dc == DC - 1))
                d = proj.tile([128, U], f16, tag=f"decP{c}")
                nc.vector.tensor_scalar_add(d[:], psd[:], bdec_t[:, c:c + 1])
                decP.append(d)
                du = proj.tile([128, U], f16, tag=f"decPu{c}")
                nc.vector.tensor_scalar(du[:], psd[:], bdec_t[:, c:c + 1],
                                        1.0 / ALPHA,
                                        mybir.AluOpType.add,
                                        mybir.AluOpType.mult)
                decPu.append(du)

            # fp16 copy of scaled encP for the EW GEMM
            eP16 = proj.tile([128, JC, TC], f16, tag="eP16")
            for c in range(JC):
                nc.vector.tensor_copy(eP16[:, c, :], encP[c][:])

            # ---- FW rows: -A*EW over fp8-path groups, fp8 hi/lo ----
            psf = psB.tile([128, V], f32, tag="ps")
            for vh in range(2):
                vsl = slice(vh * 512, (vh + 1) * 512)
                for c in range(JC):
                    nc.tensor.matmul(psf[:, vsl], eP16[:, c, :], wo16_t[:, c, vsl],
                                     start=(c == 0), stop=(c == JC - 1))
            fw32 = proj.tile([128, V], f32, tag="fw32")
            nc.vector.tensor_scalar_mul(fw32[:], psf[:], -1.0)
            fwhi = proj.tile([128, V], fp8, tag="fwhi")
            nc.vector.tensor_copy(fwhi[:], fw32[:])
            fwrem = proj.tile([128, V], f32, tag="fwrem")
            nc.vector.tensor_sub(fwrem[:], fw32[:], fwhi[:])
            fwlo = proj.tile([128, V], fp8, tag="fwlo")
            nc.vector.tensor_scalar_mul(fwlo[:], fwrem[:], 16.0)

            # ---- GWb2 = [A*DW + b_out] twice (drain operand, fp8 path) ----
            psg = psB.tile([128, V], f32, tag="ps")
            for vh in range(2):
                vsl = slice(vh * 512, (vh + 1) * 512)
                for c in range(JC):
                    nc.tensor.matmul(psg[:, vsl], decP[c][:], wo16_t[:, c, vsl],
                                     start=(c == 0), stop=(c == JC - 1))
            gwb2 = proj.tile([128, 2, V], f32, tag="gwb2")
            nc.vector.tensor_add(gwb2[:, 0, :], psg[:], bout_t[:, 0, :])
            nc.vector.tensor_copy(gwb2[:, 1, :], gwb2[:, 0, :])

            # ---- pre-init lattice pool: slot 5 of each half holds the FW
            # stationary consts (1.0@p0, 1/16@p1, zeros) ----
            for _ in range(NB_LAT):
                lt = latp.tile([128, 2, 6, U], fp8, tag="lat")
                nc.sync.dma_start(lt[:, 0, 5, :], latc_d.ap()[:])
                nc.sync.dma_start(lt[:, 1, 5, :], latc_d.ap()[:])

            nc.sync.dma_start(m_t[0][0:1, 1, :, :], fwhi[0:G, :])
            nc.sync.dma_start(m_t[0][1:2, 1, :, :], fwlo[0:G, :])

            # ---- main loop (drains lag MMs by 3 t's so the DVE queue
            # never head-of-line blocks on an unfinished psum) ----
            pend = []

            def drain():
                t_, ps_, was8 = pend.pop(0)
                osb = osbp.tile([128, V], f32, tag="osb", name="osb")
                if was8:
                    nc.vector.tensor_sub(osb[:], gwb2[:, 0, :], ps_[:])
                else:
                    nc.vector.tensor_add(osb[:], ps_[:], bout_t[:, 0, :])
                nc.sync.dma_start(out_ap[t_], osb[:])

            for g in range(NG):
                fp8_path = g < NF_FP8
                if g + 1 < NF_FP8:
                    mnext = m_t[(g + 1) % 2]
                    rs = slice((g + 1) * G, (g + 2) * G)
                    nc.sync.dma_start(mnext[0:1, 1, :, :], fwhi[rs, :])
                    nc.sync.dma_start(mnext[1:2, 1, :, :], fwlo[rs, :])
                mcur = m_t[g % 2]
                for p in range(G // 2):
                    if fp8_path:
                        xp = xpp.tile([128, 2, JC * U], f16, tag="xp")
                        for i2 in range(2):
                            t = g * G + 2 * p + i2
                            for c in range(JC):
                                nc.vector.tensor_scalar_add(
                                    xp[:, i2, c * U:(c + 1) * U], decP[c][:],
                                    encP[c][:, t:t + 1])
                        jt = jtp.tile([128, 2 * JC * U], f16, tag="jt")
                        nc.scalar.activation(jt[:], xp[:], Tanh,
                                             scale=1.0 / ALPHA)
                        rs16 = rsp.tile([128, 2 * JC * U], f16, tag="rs")
                        nc.vector.tensor_sub(rs16[:], xp[:], jt[:])
                        lat = latp.tile([128, 2, 6, U], fp8, tag="lat")
                        nc.scalar.activation(lat[:, :, 0:5, :], rs16[:], Copy)
                        for i2 in range(2):
                            t = g * G + 2 * p + i2
                            i = 2 * p + i2
                            ps = psB.tile([128, V], f32, tag="ps")
                            for c3 in range(3):
                                lhs = lat[:, i2, 2 * c3:2 * c3 + 2, :]
                                for vh in range(2):
                                    vsl = slice(vh * 512, (vh + 1) * 512)
                                    rhs = (wo8a_t[:, :, vsl] if c3 == 0 else
                                           wo8b_t[:, :, vsl] if c3 == 1 else
                                           mcur[:, :, i, vsl])
                                    nc.tensor.matmul(ps[:, vsl], lhs, rhs,
                                                     start=(c3 == 0),
                                                     stop=(c3 == 2),
                                                     perf_mode=DR)
                            pend.append((t, ps, True))
                            while len(pend) > 3:
                                drain()
                    else:
                        jbs = []
                        for i2 in range(2):
                            t = g * G + 2 * p + i2
                            jb = jbp.tile([128, JC * U], f16, tag="jb")
                            for c in range(JC):
                                nc.scalar.activation(
                                    jb[:, c * U:(c + 1) * U], decPu[c][:],
                                    Tanh, bias=encPu[c][:, t:t + 1])
                            jbs.append(jb)
                        for i2 in range(2):
                            t = g * G + 2 * p + i2
                            jb = jbs[i2]
                            ps = psB.tile([128, V], f32, tag="ps")
                            for c in range(JC):
                                for vh in range(2):
                                    vsl = slice(vh * 512, (vh + 1) * 512)
                                    nc.tensor.matmul(
                                        ps[:, vsl], jb[:, c * U:(c + 1) * U],
                                        wo16_t[:, c, vsl],
                                        start=(c == 0), stop=(c == JC - 1))
                            pend.append((t, ps, False))
                            while len(pend) > 3:
                                drain()
            while pend:
                drain()

    nc.compile()
    return nc


def _host_prep(enc_out, pred_out, W_enc, b_enc, W_dec, b_dec, W_out, b_out):
    import concourse.mybir as mybir
    np_f16 = np.dtype(np.float16)
    np_fp8 = np.dtype(mybir.dt.np(mybir.dt.float8e4))

    wencT = np.ascontiguousarray(
        (np.asarray(W_enc, np.float32) * ALPHA).T).astype(np_f16)
    wdecT = np.ascontiguousarray(
        (np.asarray(W_dec, np.float32) * ALPHA).T).astype(np_f16)
    woT = np.ascontiguousarray(np.asarray(W_out, np.float32).T)      # [J, V]

    wo16 = np.ascontiguousarray(
        woT.reshape(JC, 128, V).transpose(1, 0, 2)).astype(np_f16)   # [128,JC,V]
    wo8 = woT.astype(np_fp8)                                         # [J, V]
    wo8a = np.ascontiguousarray(
        wo8[0:256].reshape(2, 128, V).transpose(1, 0, 2))            # [128,2,V]
    wo8b = np.ascontiguousarray(
        wo8[256:512].reshape(2, 128, V).transpose(1, 0, 2))
    mstat = np.zeros((128, 2, G, V), np_fp8)
    mstat[:, 0, :, :] = wo8[512:640][:, None, :]                     # W c4 x G
    latc = np.zeros((128, U), np_fp8)
    latc[0, :] = 1.0
    latc[1, :] = 0.0625

    bencr = np.ascontiguousarray(
        (np.asarray(b_enc, np.float32) * ALPHA).reshape(JC, 128).T)
    bencu = np.ascontiguousarray(
        np.asarray(b_enc, np.float32).reshape(JC, 128).T)
    bdecr = np.ascontiguousarray(
        (np.asarray(b_dec, np.float32) * ALPHA).reshape(JC, 128).T)
    boutr = np.ascontiguousarray(np.broadcast_to(
        np.asarray(b_out, np.float32), (128, 2, V)))

    in_maps = []
    for k in range(NCORES):
        b, th = k // 2, (k % 2) * TC
        encT = np.ascontiguousarray(
            np.asarray(enc_out[b, th:th + TC], np.float32).T).astype(np_f16)
        predT = np.ascontiguousarray(
            np.asarray(pred_out[b], np.float32).T).astype(np_f16)
        in_maps.append({
            "enct": encT, "predt": predT, "wenct": wencT, "wdect": wdecT,
            "wo8a": wo8a, "wo8b": wo8b, "mstat": mstat, "latc": latc,
            "wo16": wo16, "bencr": bencr, "bencu": bencu, "bdecr": bdecr,
            "boutr": boutr,
        })
    return in_maps


def kernel(enc_out, pred_out, W_enc, b_enc, W_dec, b_dec, W_out, b_out):
    from concourse import bass_utils

    if "nc" not in _CACHE:
        _CACHE["nc"] = _build_bass()
    nc = _CACHE["nc"]

    in_maps = _host_prep(enc_out, pred_out, W_enc, b_enc, W_dec, b_dec,
                         W_out, b_out)

    trace = bool(int(os.environ.get("TRNK_PROFILE", "0")))
    res = bass_utils.run_bass_kernel_spmd(
        nc, in_maps, core_ids=list(range(NCORES)), trace=trace)
    kernel.last_exec_ns = res.exec_time_ns

    full = np.empty((B, T, U, V), np.float32)
    for k in range(NCORES):
        b, th = k // 2, (k % 2) * TC
        full[b, th:th + TC] = res.results[k]["out"]
    return full


kernel.last_exec_ns = None


# revision 18
# speedup vs baseline: 4.3863x; 1.1405x over previous
"""RNN-T joint network kernel for 8 Trainium2 NeuronCores.

Reference computation:
    enc_proj = enc_out @ W_enc.T + b_enc          # [B,T,J]
    dec_proj = pred_out @ W_dec.T + b_dec         # [B,U,J]
    joint    = tanh(enc_proj[:,:,None,:] + dec_proj[:,None,:,:])
    out      = joint @ W_out.T + b_out            # [B,T,U,V]

Shapes (hardcoded): B=4, T=256, U=128, D=512, J=640, V=1024.

Sharding: data-parallel over the B*T = 1024 encoder rows; core k gets
batch b=k//2 and t-range [(k%2)*128, (k%2)*128+128).  Each core computes
its full [128, 128, 1024] output slab.

On-device layout (per core): everything is kept j-major ("transposed")
so that the J=640 contraction of the dominant GEMM lands on the PE
partition axis:
    encP[j, t]  (5 chunks of 128 j)   decP[j, u] (+ combined bias)
    jointT[j, (t,u)] = tanh(decP[j,u] + encP[j,t])   (DVE bcast-add + ACT tanh)
    out[(t,u), v] = jointT.T @ W_outT (+ b_out via DVE on PSUM->SBUF copy)

Perf note (measured on hw): this bf16 formulation is PE-bound at ~89% of
the bf16 matmul roofline (281us busy / 308us span).  An fp8-e4m3
DoubleRow variant (2x PE throughput via a linear-residual decomposition
tanh(x) = a*x - r(x) that keeps the quantized operand small) was built
and validated (rel err 1.14e-2), but the extra per-element lattice work
it requires (x-build broadcast-adds, residual subtract, fp8 cast, plus
the unavoidable f32 PSUM drain) lands on DVE/ACT, which on this stack
run at ~1 elem/lane/cycle with ~50-300ns/op overheads: every balanced
fp8/fp16 hybrid measured 350-460us - worse than this PE-bound version.
"""

import os
import numpy as np

B, T, U, D, J, V = 4, 256, 128, 512, 640, 1024
NCORES = 8
TC = (B * T) // NCORES          # 128 t-rows per core
JC = J // 128                   # 5 j-chunks
DC = D // 128                   # 4 d-chunks
G = 8                            # t-rows per lattice group
NG = TC // G                    # 16 groups

# matmul dtype for the dominant GEMM: "bfloat16", "float32", "float32r"
MAIN_DT_NAME = os.environ.get("TRNK_DT", "bfloat16")

_CACHE = {}


def _build_bass():
    import concourse.mybir as mybir
    import concourse.tile as tile
    import concourse.bacc as bacc

    f32 = mybir.dt.float32
    main_dt = getattr(mybir.dt, MAIN_DT_NAME)
    # projections / DVE-visible tiles: bf16 in bf16 mode, else plain f32
    bf16_mode = MAIN_DT_NAME == "bfloat16"
    proj_dt = mybir.dt.bfloat16 if bf16_mode else f32
    lat_dt = proj_dt            # dtype of decP/encP/pre tiles (DVE ops)

    nc = bacc.Bacc("TRN2", debug=False)

    enc_d = nc.dram_tensor("enct", [D, TC], proj_dt, kind="ExternalInput")
    pred_d = nc.dram_tensor("predt", [D, U], proj_dt, kind="ExternalInput")
    wenc_d = nc.dram_tensor("wenct", [D, J], proj_dt, kind="ExternalInput")
    wdec_d = nc.dram_tensor("wdect", [D, J], proj_dt, kind="ExternalInput")
    wout_d = nc.dram_tensor("woutt", [J, V], main_dt, kind="ExternalInput")
    bcomb_d = nc.dram_tensor("bcomb", [128, JC], f32, kind="ExternalInput")
    bout_d = nc.dram_tensor("boutr", [128, V], f32, kind="ExternalInput")
    out_d = nc.dram_tensor("out", [TC, U, V], f32, kind="ExternalOutput")

    enc_ap, pred_ap = enc_d.ap(), pred_d.ap()
    wenc_ap, wdec_ap, wout_ap = wenc_d.ap(), wdec_d.ap(), wout_d.ap()
    out_ap = out_d.ap()

    Tanh = mybir.ActivationFunctionType.Tanh
    Add = mybir.AluOpType.add

    with tile.TileContext(nc) as tc:
        with (
            tc.tile_pool(name="consts", bufs=1) as consts,
            tc.tile_pool(name="proj", bufs=1) as proj,
            tc.tile_pool(name="joint", bufs=2 * JC) as jointp,
            tc.tile_pool(name="osb", bufs=6) as osbp,
            tc.tile_pool(name="psB", bufs=4, space="PSUM") as psB,
        ):
            # ---- load inputs; projection operands first so PE can start ----
            enc_t, pred_t, wenc_t, wdec_t = [], [], [], []
            for dc in range(DC):
                sl = slice(dc * 128, (dc + 1) * 128)
                a = consts.tile([128, TC], proj_dt, tag=f"enc{dc}")
                nc.sync.dma_start(a[:], enc_ap[sl, :])
                enc_t.append(a)
                p = consts.tile([128, U], proj_dt, tag=f"pred{dc}")
                nc.sync.dma_start(p[:], pred_ap[sl, :])
                pred_t.append(p)
                we = consts.tile([128, J], proj_dt, tag=f"wenc{dc}")
                nc.sync.dma_start(we[:], wenc_ap[sl, :])
                wenc_t.append(we)
                wd = consts.tile([128, J], proj_dt, tag=f"wdec{dc}")
                nc.sync.dma_start(wd[:], wdec_ap[sl, :])
                wdec_t.append(wd)

            bcomb_t = consts.tile([128, JC], f32, tag="bcomb")
            nc.sync.dma_start(bcomb_t[:], bcomb_d.ap()[:])
            wout_t = []
            for c in range(JC):
                w = consts.tile([128, V], main_dt, tag=f"wout{c}")
                nc.sync.dma_start(w[:], wout_ap[c * 128:(c + 1) * 128, :])
                wout_t.append(w)
            bout_t = consts.tile([128, V], f32, tag="bout")
            nc.sync.dma_start(bout_t[:], bout_d.ap()[:])

            # ---- projections: encP[c][j, t], decP[c][j, u] (bias folded) ----
            encP, decP = [], []
            for c in range(JC):
                jsl = slice(c * 128, (c + 1) * 128)
                pse = psB.tile([128, TC], f32, tag="ps")
                for dc in range(DC):
                    nc.tensor.matmul(pse[:], wenc_t[dc][:, jsl], enc_t[dc][:],
                                     start=(dc == 0), stop=(dc == DC - 1))
                # encP stays f32: ScalarE's bias operand must be f32.
                # Copies ride DVE so ACT's FIFO is free for early tanh ops.
                e = proj.tile([128, TC], f32, tag=f"encP{c}")
                nc.vector.tensor_copy(e[:], pse[:])
                encP.append(e)

                psd = psB.tile([128, U], f32, tag="ps")
                for dc in range(DC):
                    nc.tensor.matmul(psd[:], wdec_t[dc][:, jsl], pred_t[dc][:],
                                     start=(dc == 0), stop=(dc == DC - 1))
                d = proj.tile([128, U], lat_dt, tag=f"decP{c}")
                nc.vector.tensor_scalar_add(d[:], psd[:], bcomb_t[:, c:c + 1])
                decP.append(d)

            # ---- main loop over t-groups ----
            for g in range(NG):
                # joint[j, (i,u)] = tanh(decP[j,u] + encP[j,t]) — the
                # broadcast-add rides ScalarE's per-partition bias port.
                # Emit t-major so each t's matmuls unlock after JC ACT ops,
                # not after (JC-1)*G+1 of them.
                joint_t = []
                jview = []
                for c in range(JC):
                    jt = jointp.tile([128, G * U], main_dt, tag="joint")
                    joint_t.append(jt)
                    jview.append(jt[:] if main_dt == proj_dt
                                 else jt.bitcast(proj_dt)[:])
                for i in range(G):
                    t = g * G + i
                    for c in range(JC):
                        nc.scalar.activation(
                            jview[c][:, i * U:(i + 1) * U], decP[c][:], Tanh,
                            bias=encP[c][:, t:t + 1])

                for i in range(G):
                    t = g * G + i
                    usl = slice(i * U, (i + 1) * U)
                    osb = osbp.tile([128, V], f32, tag="osb")
                    ps = psB.tile([128, V], f32, tag="ps")
                    for v in range(2):
                        vsl = slice(v * 512, (v + 1) * 512)
                        for c in range(JC):
                            nc.tensor.matmul(ps[:, vsl], joint_t[c][:, usl],
                                             wout_t[c][:, vsl],
                                             start=(c == 0), stop=(c == JC - 1))
                    if g == NG - 1 and i >= G - 2:
                        for v in range(2):
                            vsl = slice(v * 512, (v + 1) * 512)
                            nc.vector.tensor_add(osb[:, vsl], ps[:, vsl],
                                                 bout_t[:, vsl])
                            nc.sync.dma_start(out_ap[t][:, vsl], osb[:, vsl])
                    else:
                        nc.vector.tensor_add(osb[:], ps[:], bout_t[:])
                        nc.sync.dma_start(out_ap[t], osb[:])

    nc.compile()
    return nc


def _host_prep(enc_out, pred_out, W_enc, b_enc, W_dec, b_dec, W_out, b_out):
    import concourse.mybir as mybir
    main_np = np.dtype(mybir.dt.np(getattr(mybir.dt, MAIN_DT_NAME)))
    proj_np = main_np if MAIN_DT_NAME == "bfloat16" else np.dtype(np.float32)

    wencT = np.ascontiguousarray(np.asarray(W_enc, np.float32).T).astype(proj_np)
    wdecT = np.ascontiguousarray(np.asarray(W_dec, np.float32).T).astype(proj_np)
    woutT = np.ascontiguousarray(np.asarray(W_out, np.float32).T).astype(main_np)
    bcomb = np.ascontiguousarray(
        (np.asarray(b_enc, np.float32) + np.asarray(b_dec, np.float32))
        .reshape(JC, 128).T)
    boutr = np.ascontiguousarray(
        np.broadcast_to(np.asarray(b_out, np.float32), (128, V)))

    in_maps = []
    for k in range(NCORES):
        b, th = k // 2, (k % 2) * TC
        encT = np.ascontiguousarray(
            np.asarray(enc_out[b, th:th + TC], np.float32).T).astype(proj_np)
        predT = np.ascontiguousarray(
            np.asarray(pred_out[b], np.float32).T).astype(proj_np)
        in_maps.append({
            "enct": encT, "predt": predT, "wenct": wencT, "wdect": wdecT,
            "woutt": woutT, "bcomb": bcomb, "boutr": boutr,
        })
    return in_maps


def kernel(enc_out, pred_out, W_enc, b_enc, W_dec, b_dec, W_out, b_out):
    from concourse import bass_utils

    if "nc" not in _CACHE:
        _CACHE["nc"] = _build_bass()
    nc = _CACHE["nc"]

    in_maps = _host_prep(enc_out, pred_out, W_enc, b_enc, W_dec, b_dec,
                         W_out, b_out)

    trace = bool(int(os.environ.get("TRNK_PROFILE", "0")))
    res = bass_utils.run_bass_kernel_spmd(
        nc, in_maps, core_ids=list(range(NCORES)), trace=trace)
    kernel.last_exec_ns = res.exec_time_ns

    full = np.empty((B, T, U, V), np.float32)
    for k in range(NCORES):
        b, th = k // 2, (k % 2) * TC
        full[b, th:th + TC] = res.results[k]["out"]
    return full


kernel.last_exec_ns = None
